# revision 1
# baseline (speedup 1.0000x reference)
"""Trainium2 Bass kernel for nn_Decoder (dense_transformer).

Model: small conv/FC "StateNet" produces a per-batch query vector q [B,128];
3 decoder blocks do single-query cross-attention against enc_out [B,512,128]
plus an FFN; final layernorm + head -> [B, 37].

Key algebraic restructuring (cuts dominant FLOPs ~16x vs materializing K/V):
  - enc_out is layernormalized ONCE (no gamma/beta): nrm = (x-m)/std.
    Per-layer ln1/ln2 gamma/beta fold into the small matrices:
      scores: s[b,h,t] = sum_e nrm[b,t,e] * U'[e,(h,b)],
        U'[e,(h,b)] = g1[e] * sum_d wk[e,(h,d)] Q[b,(h,d)]
        (the beta1 term is constant over t -> drops out of softmax)
      values: att[b,(h,d)] = sum_e rn[b,h,e] * (g2[e]*wv[e,(h,d)]) + c2[(h,d)],
        rn[b,h,:] = sum_t wei[b,h,t] * nrm[b,t,:],  c2 = b2 @ wv  (sum_t wei = 1)
  - softmax without max-subtraction (scores are O(1); exp is safe in fp32).
  - big-tensor matmul operands in bf16 (1 cy/row on PE vs 4 for fp32);
    the q-side residual stream stays fp32.

Sharding: pure data parallel, batch 2048 -> 8 cores x 256. Parameters
replicated. Output gathered by concatenation.
"""

import math
from contextlib import ExitStack

import numpy as np

import concourse.bass as bass
import concourse.tile as tile
from concourse import bacc, mybir
from concourse.bass_utils import run_bass_kernel_spmd
from concourse.masks import make_identity

F32 = mybir.dt.float32
BF16 = mybir.dt.bfloat16
AF = mybir.ActivationFunctionType
OP = mybir.AluOpType

P = 128
T = 512
E = 128
H = 8
D = 16
L = 3
NCH = T // P          # 4 t-chunks per batch element
BN_S = 1.0 / math.sqrt(1.0 + 1e-5)   # eval-mode BatchNorm scale
EPS = 1e-5
N_CORES = 8
B_FULL = 2048

SHARDED = ("enc_out", "x1", "x2", "x3")


def _ap(t, offset, pattern):
    return bass.AP(tensor=t.tensor, offset=offset, ap=[list(p) for p in pattern])


def decoder_body(ctx: ExitStack, tc: tile.TileContext, out_ap: bass.AP,
                 ins: dict, BC: int, stage: int = 99):
    # stage (debug bisect): 1=statenet 2=+norm 3=+qside 4=+attn 5=+ffn 99=full
    nc = tc.nc

    def dbg_out(tag_ap):
        nc.sync.dma_start(out=out_ap.rearrange("b o -> o b"),
                          in_=tag_ap[0:37, 0:BC])
    SG = min(64, BC)          # supergroup batch size held in SBUF
    NSG = BC // SG
    assert SG % 16 == 0 and BC % SG == 0
    NG16 = SG // 16
    NB = (BC + P - 1) // P    # 128-wide batch tiles for statenet

    dma = nc.sync.dma_start

    # ---------------- pools live for the whole kernel ----------------
    const = ctx.enter_context(tc.tile_pool(name="const", bufs=1))
    perm = ctx.enter_context(tc.tile_pool(name="perm", bufs=1))
    wp = ctx.enter_context(tc.tile_pool(name="wp", bufs=1))
    work = ctx.enter_context(tc.tile_pool(name="work", bufs=3))
    # PSUM: 8 banks total; one tag per pool, tiles <= 1 bank (2KB/partition)
    p_sc = ctx.enter_context(tc.tile_pool(name="p_sc", bufs=2, space="PSUM"))
    p_tr = ctx.enter_context(tc.tile_pool(name="p_tr", bufs=2, space="PSUM"))
    p_v = ctx.enter_context(tc.tile_pool(name="p_v", bufs=2, space="PSUM"))
    p_q = ctx.enter_context(tc.tile_pool(name="p_q", bufs=2, space="PSUM"))

    def psum(pool, shape, dt_=F32):
        return pool.tile(shape, dt_, tag=pool.name, name=pool.name + "_t")

    id_f32 = const.tile([P, P], F32)
    id_bf = const.tile([P, P], BF16)
    make_identity(nc, id_f32[:])
    make_identity(nc, id_bf[:])
    ones_col = const.tile([P, 1], F32)     # lhsT for partition-sums
    nc.vector.memset(ones_col[:], 1.0)
    ones_row = const.tile([1, P], F32)     # lhsT for partition-broadcast
    nc.vector.memset(ones_row[:], 1.0)
    ones_cbf = const.tile([P, 1], BF16)    # bf16 lhsT for exp-sum reduction
    nc.vector.memset(ones_cbf[:], 1.0)
    eps_t = const.tile([P, 1], F32)
    nc.vector.memset(eps_t[:], EPS)

    q_T = perm.tile([P, BC], F32)          # persistent residual stream [E, b]
    attT = perm.tile([P, SG], F32)
    U_sb = perm.tile([P, H, SG], BF16)     # U'[e, h, b_local]
    hg_sb = perm.tile([P, 4, SG], F32)     # gelu(ff1) chunks

    # =======================================================================
    # StateNet -> q0   (tiny; scoped pool so its SBUF is reclaimed)
    # =======================================================================
    with tc.tile_pool(name="snet", bufs=1) as sn:
        # ---- transposed inputs x1T [111, BC], x2T [28, BC], x3T [4, BC]
        x1T = sn.tile([111, BC], F32)
        x2T = sn.tile([28, BC], F32)
        cat64 = sn.tile([64, BC], F32)   # h1
        cat16 = sn.tile([16, BC], F32)   # h2
        x3c = sn.tile([4, BC], F32)      # x3  (all separate base-0 tiles --
        x3T = x3c[0:4, :]                # nonzero-base matmuls crash on hw)
        x1_f = ins["x1"].rearrange("b c h w -> b (c h w)")
        x2_f = ins["x2"].rearrange("b c h w -> b (c h w)")
        for i in range(NB):
            n = min(P, BC - i * P)
            for (srcx, dstT, w) in ((x1_f, x1T[:], 111), (x2_f, x2T[:], 28),
                                    (ins["x3"], x3T, 4)):
                xin = sn.tile([P, w], F32, tag="xin")
                dma(out=xin[:n, :], in_=srcx[i * P:i * P + n, :])
                pst = psum(p_q, [w, P])
                nc.tensor.transpose(pst[:, :n], xin[:n, :], id_f32[:n, :n])
                nc.scalar.copy(dstT[:, i * P:i * P + n], pst[:, :n])

        # ---- conv weights: [o, (c,kh)] with BN gamma' folded; then transpose
        def conv_w(dram_ap, O_, C_, gname, bname, cbname):
            KK = C_ * 3
            ws = sn.tile([O_, KK], F32, tag="ws" + gname)
            dma(out=ws[:], in_=_ap(dram_ap, 1, [[C_ * 9, O_], [9, C_], [3, 3]]))
            g = sn.tile([O_, 1], F32, tag="g" + gname)
            dma(out=g[:], in_=ins[gname])
            gp = sn.tile([O_, 1], F32, tag="gp" + gname)
            nc.scalar.mul(gp[:], g[:], BN_S)
            cb = sn.tile([O_, 1], F32, tag="cb" + gname)
            dma(out=cb[:], in_=ins[cbname])
            bb = sn.tile([O_, 1], F32, tag="bb" + gname)
            dma(out=bb[:], in_=ins[bname])
            beff = sn.tile([O_, 1], F32, tag="be" + gname)
            nc.vector.tensor_mul(beff[:], cb[:], gp[:])
            nc.vector.tensor_add(beff[:], beff[:], bb[:])
            wsc = sn.tile([O_, KK], F32, tag="wsc" + gname)
            nc.vector.tensor_scalar_mul(wsc[:], ws[:], gp[:])
            pswt = psum(p_q, [KK, O_])
            nc.tensor.transpose(pswt[:], wsc[:], id_f32[:O_, :O_])
            wT = sn.tile([KK, O_], F32, tag="wT" + gname)
            nc.scalar.copy(wT[:], pswt[:])
            return wT, beff

        w1T, b1e = conv_w(ins["c11_w"], 8, 3, "bn11_g", "bn11_b", "c11_b")
        w2T, b2e = conv_w(ins["c12_w"], 8, 8, "bn12_g", "bn12_b", "c12_b")
        w3T, b3e = conv_w(ins["c21_w"], 8, 7, "bn21_g", "bn21_b", "c21_b")

        # ---- im2col: srcT [C*W, BC] -> rhs [(c,kh), W, BC] (zero padded)
        def im2col(srcT, C_, W_, tag):
            rhs = sn.tile([24, 37, BC], F32, tag="im", name="imt")[
                :C_ * 3, :W_, :]
            nc.vector.memset(rhs[:], 0.0)
            for c in range(C_):
                for kh in range(3):
                    lo = max(0, 1 - kh)
                    hi = min(W_, W_ + 1 - kh)
                    n = hi - lo
                    s0 = c * W_ + lo + kh - 1
                    k_ = c * 3 + kh
                    dma(out=rhs[k_:k_ + 1, lo:hi, :], in_=srcT[s0:s0 + n, :])
            return rhs

        def conv_apply(rhs, wT, beff, O_, W_, tag):
            y = sn.tile([8, 37, BC], F32, tag="yt", name="ytt")[:O_, :W_, :]
            step = max(1, 512 // BC)
            for i0 in range(0, W_, step):
                n = min(step, W_ - i0)
                psc = psum(p_q, [O_, step, BC])
                nc.tensor.matmul(psc[:, :n, :], wT[:], rhs[:, i0:i0 + n, :])
                nc.scalar.activation(y[:, i0:i0 + n, :], psc[:, :n, :],
                                     AF.Relu, bias=beff[:])
            return y

        r9 = im2col(x1T, 3, 37, "im")
        y1 = conv_apply(r9, w1T, b1e, 8, 37, "ytile")
        r24 = sn.tile([24, 37, BC], F32, tag="im", name="imt")
        nc.vector.memset(r24[:], 0.0)
        for c in range(8):
            for kh in range(3):
                lo = max(0, 1 - kh)
                hi = min(37, 37 + 1 - kh)
                n = hi - lo
                k_ = c * 3 + kh
                dma(out=r24[k_:k_ + 1, lo:hi, :],
                    in_=y1[c:c + 1, lo + kh - 1:lo + kh - 1 + n, :])
        y2 = conv_apply(r24, w2T, b2e, 8, 37, "ytile2")

        r21 = im2col(x2T, 7, 4, "im21")
        y2b = conv_apply(r21, w3T, b3e, 8, 4, "ybtile")

        # ---- reorder conv outputs to (i,o)-major rows for the FC layers
        # rows per 128-chunk are (o, i)-blocked: chunk k holds i in
        # [ilo, ihi) for all 8 channels, contiguous per channel
        y2r = []
        for k, (ilo, ihi) in enumerate(((0, 16), (16, 32), (32, 37))):
            ni = ihi - ilo
            t_ = sn.tile([ni * 8, BC], F32, tag=f"y2r{k}")
            for o in range(8):
                dma(out=t_[o * ni:(o + 1) * ni, :],
                    in_=y2[o:o + 1, ilo:ihi, :])
            y2r.append(t_)
        y2br = sn.tile([32, BC], F32)
        for o in range(8):
            dma(out=y2br[o * 4:(o + 1) * 4, :], in_=y2b[o:o + 1, :, :])

        # ---- fc1 [296 -> 64] with matching (o,i)-blocked weight rows
        ps_h1 = psum(p_v, [64, BC])
        for k, (ilo, ihi) in enumerate(((0, 16), (16, 32), (32, 37))):
            ni = ihi - ilo
            fw = sn.tile([ni * 8, 64], F32, tag=f"fw{k}")
            dma(out=fw[:], in_=_ap(ins["fc1_w"], ilo * 64,
                                   [[37 * 64, 8], [64, ni], [1, 64]]))
            nc.tensor.matmul(ps_h1[:], fw[:], y2r[k][:],
                             start=(k == 0), stop=(k == 2))
        fb1 = sn.tile([64, 1], F32)
        dma(out=fb1[:], in_=ins["fc1_b"])
        h1 = cat64[0:64, :]
        nc.scalar.activation(h1, ps_h1[:], AF.Relu, bias=fb1[:])

        # ---- fc2 [32 -> 16]
        fw2 = sn.tile([32, 16], F32)
        dma(out=fw2[:], in_=ins["fc2_w"])   # rows (o,i) = natural order
        ps_h2 = psum(p_v, [16, BC])
        nc.tensor.matmul(ps_h2[:], fw2[:], y2br[:])
        fb2 = sn.tile([16, 1], F32)
        dma(out=fb2[:], in_=ins["fc2_b"])
        h2 = cat16[0:16, :]
        nc.scalar.activation(h2, ps_h2[:], AF.Relu, bias=fb2[:])

        # ---- fc [84 -> 128] on concat(h1, h2, x3)
        fcw64 = sn.tile([64, E], F32)
        dma(out=fcw64[:], in_=ins["fc_w"][0:64, :])
        fcw16 = sn.tile([16, E], F32)
        dma(out=fcw16[:], in_=ins["fc_w"][64:80, :])
        fcw3 = sn.tile([4, E], F32)
        dma(out=fcw3[:], in_=ins["fc_w"][80:84, :])
        ps_q0 = psum(p_v, [P, BC])
        nc.tensor.matmul(ps_q0[:], fcw64[:], cat64[:], start=True, stop=False)
        nc.tensor.matmul(ps_q0[:], fcw16[:], cat16[:], start=False, stop=False)
        nc.tensor.matmul(ps_q0[:], fcw3[:], x3c[:], start=False, stop=True)
        fcb = sn.tile([P, 1], F32)
        dma(out=fcb[:], in_=ins["fc_b"])
        nc.scalar.activation(q_T[:], ps_q0[:], AF.Relu, bias=fcb[:])

    if stage <= 1:
        dbg_out(q_T)
        return

    # =======================================================================
    # helper: layernorm of a feature-major [128, n] fp32 slice (stats over
    # partitions via PE ones-matmuls; PE broadcast back). Returns (x-m)*rstd.
    # =======================================================================
    def ln_cols(x_sl, n, tag):
        sq = work.tile([P, BC], F32, tag="sq", name="sq")[:, :n]
        nc.vector.tensor_mul(sq[:], x_sl, x_sl)
        ps_st = psum(p_q, [1, 2 * n])
        nc.tensor.matmul(ps_st[:, 0:n], ones_col[:], x_sl)
        nc.tensor.matmul(ps_st[:, n:2 * n], ones_col[:], sq[:])
        mm_row = work.tile([1, 2 * BC], F32, tag="mmr", name="mmr")[:, :2 * n]
        nc.scalar.mul(mm_row[:], ps_st[:], 1.0 / E)
        var = work.tile([1, BC], F32, tag="var", name="var")[:, :n]
        nc.vector.tensor_mul(var[:], mm_row[:, 0:n], mm_row[:, 0:n])
        nc.vector.tensor_tensor(var[:], mm_row[:, n:2 * n], var[:],
                                op=OP.subtract)
        srt = work.tile([1, BC], F32, tag="srt", name="srt")[:, :n]
        nc.scalar.activation(srt[:], var[:], AF.Sqrt, bias=eps_t[0:1, :])
        nc.vector.reciprocal(srt[:], srt[:])
        ps_b = psum(p_q, [P, 2 * n])
        nc.tensor.matmul(ps_b[:, 0:n], ones_row[:], mm_row[:, 0:n])
        nc.tensor.matmul(ps_b[:, n:2 * n], ones_row[:], srt[:])
        xo = work.tile([P, BC], F32, tag="xo", name="xo")[:, :n]
        nc.vector.tensor_tensor(xo[:], x_sl, ps_b[:, 0:n], op=OP.subtract)
        nc.vector.tensor_tensor(xo[:], xo[:], ps_b[:, n:2 * n], op=OP.mult)
        return xo

    def load_col(name, l, n, tag):
        t_ = wp.tile([n, 1], F32, tag=tag)
        src = ins[name]
        dma(out=t_[:], in_=src[l] if l is not None else src)
        return t_

    # =======================================================================
    # main: supergroups of SG batch elements
    # =======================================================================
    with tc.tile_pool(name="nrm", bufs=1) as nrmp:
        nrm_nat = nrmp.tile([P, SG, NCH, E], BF16)   # [t%128, b, t//128, e]
        nrm_T = nrmp.tile([P, SG, T], BF16)          # [e, b, t]

        for sg in range(NSG):
            b0 = sg * SG
            # ---------------- stage N: normalize enc_out once ----------------
            for j in range(SG):
                enc_t = work.tile([P, NCH, E], F32, tag="enc")
                dma(out=enc_t[:],
                    in_=ins["enc_out"][b0 + j].rearrange("(c p) e -> p c e", p=P))
                st = work.tile([P, NCH, 6], F32, tag="st")
                mv = work.tile([P, NCH, 2], F32, tag="mv")
                for c in range(NCH):
                    nc.vector.bn_stats(st[:, c, :], enc_t[:, c, :])
                    nc.vector.bn_aggr(mv[:, c, :], st[:, c, :])
                rr = work.tile([P, NCH], F32, tag="rr")
                nc.scalar.activation(rr[:], mv[:, :, 1], AF.Sqrt,
                                     bias=eps_t[:])
                nc.vector.reciprocal(rr[:], rr[:])
                for c in range(NCH):
                    nc.vector.tensor_scalar(
                        nrm_nat[:, j, c, :], enc_t[:, c, :],
                        mv[:, c, 0:1], rr[:, c:c + 1],
                        op0=OP.subtract, op1=OP.mult)
                ps_t = psum(p_tr, [P, T], BF16)
                for c in range(NCH):
                    nc.tensor.transpose(ps_t[:, c * P:(c + 1) * P],
                                        nrm_nat[:, j, c, :], id_bf[:])
                nc.scalar.copy(nrm_T[:, j, :], ps_t[:])

            if stage <= 2:
                continue
            # ---------------- 3 decoder layers for this supergroup ----------
            sl = q_T[:, b0:b0 + SG]
            for l in range(L):
                # ---- per-layer weights + folds
                wq_t = wp.tile([E, E], F32, tag="wq")
                dma(out=wq_t[:], in_=ins["wq"][l])
                wk_t = wp.tile([E, E], F32, tag="wk")
                dma(out=wk_t[:], in_=ins["wk"][l])
                wv_t = wp.tile([E, E], F32, tag="wv")
                dma(out=wv_t[:], in_=ins["wv"][l])
                pj_t = wp.tile([E, E], F32, tag="pj")
                dma(out=pj_t[:], in_=ins["proj_w"][l])
                pj_spl = wp.tile([16, H, E], F32, tag="pjspl")
                dma(out=pj_spl[:], in_=_ap(ins["proj_w"], l * E * E,
                                           [[E, 16], [16 * E, H], [1, E]]))
                f1_t = wp.tile([E, 4 * E], F32, tag="f1")
                dma(out=f1_t[:], in_=ins["ff_w1"][l])
                f2_t = wp.tile([P, 4, E], F32, tag="f2")
                dma(out=f2_t[:], in_=ins["ff_w2"][l].rearrange(
                    "(c p) e -> p c e", p=P))
                g1 = load_col("ln1_g", l, P, "g1")
                g2 = load_col("ln2_g", l, P, "g2")
                b2 = load_col("ln2_b", l, P, "b2")
                g3 = load_col("ln3_g", l, P, "g3")
                b3 = load_col("ln3_b", l, P, "b3")
                g4 = load_col("ln4_g", l, P, "g4")
                b4 = load_col("ln4_b", l, P, "b4")
                pjb = load_col("proj_b", l, P, "pjb")
                fb2_ = load_col("ff_b2", l, P, "fb2")
                fb1_ = wp.tile([P, 4], F32, tag="fb1")
                dma(out=fb1_[:], in_=ins["ff_b1"][l].rearrange(
                    "(c p) -> p c", p=P))

                wq_e = wp.tile([E, E], F32, tag="wqe")
                nc.vector.tensor_scalar_mul(wq_e[:], wq_t[:], g3[:])
                # per-head split layouts [16d, 8h, ...] -- engines can only
                # address partitions at 32-aligned bases, so every per-head
                # operand must live base-0 in a split tile.
                qb_ps = psum(p_q, [16, H])
                for h in range(H):
                    nc.tensor.matmul(qb_ps[:, h:h + 1],
                                     wq_t[:, 16 * h:16 * h + 16], b3[:])
                qb_spl = wp.tile([16, H], F32, tag="qbspl")
                nc.scalar.copy(qb_spl[:], qb_ps[:])
                wk_spl = wp.tile([16, H, E], F32, tag="wkspl")
                for hh in range(2):
                    ps_kT = psum(p_v, [16, 4, E])
                    for h4 in range(4):
                        h = hh * 4 + h4
                        nc.tensor.transpose(ps_kT[:, h4, :],
                                            wk_t[:, 16 * h:16 * h + 16],
                                            id_f32[:])
                    nc.scalar.copy(wk_spl[:, 4 * hh:4 * hh + 4, :], ps_kT[:])
                wv_e = wp.tile([E, E], F32, tag="wve")
                nc.vector.tensor_scalar_mul(wv_e[:], wv_t[:], g2[:])
                wv_bf = wp.tile([E, E], BF16, tag="wvbf")
                nc.vector.tensor_copy(wv_bf[:], wv_e[:])
                ps2 = psum(p_q, [P, 1])
                nc.tensor.matmul(ps2[:], wv_e[:], b2[:])
                c2 = wp.tile([P, 1], F32, tag="c2")
                nc.scalar.copy(c2[:], ps2[:])
                # fold the c2 (beta2) attention bias through proj_w:
                # q += (att0+c2)@proj + proj_b = att0@proj + (c2@proj + proj_b)
                ps2b = psum(p_q, [P, 1])
                nc.tensor.matmul(ps2b[:], pj_t[:], c2[:])
                bias2 = wp.tile([P, 1], F32, tag="bias2")
                nc.vector.tensor_add(bias2[:], ps2b[:], pjb[:])
                f1_e = wp.tile([E, 4 * E], F32, tag="f1e")
                nc.vector.tensor_scalar_mul(f1_e[:], f1_t[:], g4[:])
                ps3 = psum(p_q, [P, 4])
                for m in range(4):
                    nc.tensor.matmul(ps3[:, m:m + 1], f1_e[:, m * E:(m + 1) * E],
                                     b4[:])
                fb1e = wp.tile([P, 4], F32, tag="fb1e")
                nc.vector.tensor_add(fb1e[:], ps3[:], fb1_[:])

                # ---- q-side: ln3 -> Q -> U'
                qx3 = ln_cols(sl, SG, "l3")
                ps_Q = psum(p_v, [16, H, SG])
                for h in range(H):
                    nc.tensor.matmul(ps_Q[:, h, :],
                                     wq_e[:, 16 * h:16 * h + 16], qx3[:])
                Q_spl = work.tile([16, H, SG], F32, tag="Qspl")
                for h in range(H):
                    nc.scalar.activation(Q_spl[:, h, :], ps_Q[:, h, :],
                                         AF.Identity,
                                         bias=qb_spl[:, h:h + 1])
                ps_U = psum(p_sc, [P, H * SG])
                for h in range(H):
                    nc.tensor.matmul(ps_U[:, h * SG:(h + 1) * SG],
                                     wk_spl[:, h, :], Q_spl[:, h, :])
                nc.scalar.activation(
                    U_sb[:].rearrange("p h b -> p (h b)"), ps_U[:],
                    AF.Copy, scale=g1[:])

                if stage <= 3:
                    continue
                # ---- attention, 16 batch elements at a time.
                # scores land [t, (c,b,h)] (T on partitions), softmax-sum via
                # a PE ones-reduction; the values matmul directly produces
                # rn_T [e, (b,h)] -- no transposes needed.
                rnT_all = work.tile([P, SG, H], BF16, tag="rnTall")
                for g in range(NG16):
                    blb = g * 16
                    ps_s = psum(p_sc, [P, NCH, 16, H])
                    for bl in range(16):
                        for c in range(NCH):
                            nc.tensor.matmul(
                                ps_s[:, c, bl, :],
                                nrm_T[:, blb + bl, c * P:(c + 1) * P],
                                U_sb[:, :, blb + bl])
                    expw = work.tile([P, NCH, 16, H], BF16, tag="expw")
                    nc.scalar.activation(expw[:], ps_s[:], AF.Exp,
                                         scale=float(D ** 0.5))
                    ps_den = psum(p_q, [1, NCH * 16 * H])
                    nc.tensor.matmul(ps_den[:], ones_cbf[:],
                                     expw[:].rearrange("p c b h -> p (c b h)"))
                    den_r = work.tile([1, 16 * H], F32, tag="denr")
                    nc.vector.reduce_sum(
                        den_r[:],
                        ps_den[:].rearrange("p (c x) -> p x c", c=NCH),
                        axis=mybir.AxisListType.X)
                    nc.vector.reciprocal(den_r[:], den_r[:])
                    ps_rb = psum(p_q, [P, 16 * H])
                    nc.tensor.matmul(ps_rb[:], ones_row[:], den_r[:])
                    rb_sb = work.tile([P, 16 * H], F32, tag="rbsb")
                    nc.scalar.copy(rb_sb[:], ps_rb[:])
                    ps_v_ = psum(p_v, [P, 16, H])
                    for bl in range(16):
                        for c in range(NCH):
                            nc.tensor.matmul(
                                ps_v_[:, bl, :],
                                nrm_nat[:, blb + bl, c, :],
                                expw[:, c, bl, :],
                                start=(c == 0), stop=(c == NCH - 1))
                    nc.vector.tensor_tensor(
                        rnT_all[:, blb:blb + 16, :], ps_v_[:],
                        rb_sb[:].rearrange("p (b h) -> p b h", h=H),
                        op=OP.mult)

                # ---- att[d, h, b] for the whole supergroup, then proj
                ps_at = psum(p_sc, [16, H, SG])
                for h in range(H):
                    nc.tensor.matmul(ps_at[:, h, :],
                                     wv_bf[:, 16 * h:16 * h + 16],
                                     rnT_all[:, :, h])
                att_sb = work.tile([16, H, SG], F32, tag="attsb")
                nc.scalar.copy(att_sb[:], ps_at[:])
                ps_p = psum(p_v, [P, SG])
                for h in range(H):
                    nc.tensor.matmul(ps_p[:], pj_spl[:, h, :], att_sb[:, h, :],
                                     start=(h == 0), stop=(h == H - 1))
                tmp = work.tile([P, SG], F32, tag="tmp")
                nc.vector.tensor_scalar(tmp[:], ps_p[:], bias2[:], None,
                                        op0=OP.add)
                nc.vector.tensor_add(sl, sl, tmp[:])

                if stage <= 4:
                    continue
                # ---- FFN
                qx4 = ln_cols(sl, SG, "l4")
                for m in range(4):
                    ps_h = psum(p_v, [P, SG])
                    nc.tensor.matmul(ps_h[:], f1_e[:, m * E:(m + 1) * E], qx4[:])
                    # tanh-approx gelu (CoreSim has no Gelu/Erf; approx error
                    # ~5e-5 at these pre-activation scales)
                    gx = work.tile([P, SG], F32, tag="gx")
                    nc.scalar.activation(gx[:], ps_h[:], AF.Identity,
                                         bias=fb1e[:, m:m + 1])
                    gx2 = work.tile([P, SG], F32, tag="gx2")
                    nc.vector.tensor_mul(gx2[:], gx[:], gx[:])
                    nc.vector.tensor_scalar(gx2[:], gx2[:], 0.044715, 1.0,
                                            op0=OP.mult, op1=OP.add)
                    nc.vector.tensor_mul(gx2[:], gx2[:], gx[:])
                    gv = work.tile([P, SG], F32, tag="gv")
                    nc.scalar.activation(gv[:], gx2[:], AF.Tanh,
                                         scale=0.7978845608028654)
                    nc.vector.tensor_scalar(gv[:], gv[:], 1.0, None, op0=OP.add)
                    nc.vector.tensor_mul(gv[:], gv[:], gx[:])
                    nc.vector.tensor_scalar(hg_sb[:, m, :], gv[:], 0.5, None,
                                            op0=OP.mult)
                ps_f = psum(p_v, [P, SG])
                for k in range(4):
                    nc.tensor.matmul(ps_f[:], f2_t[:, k, :], hg_sb[:, k, :],
                                     start=(k == 0), stop=(k == 3))
                tmp2 = work.tile([P, SG], F32, tag="tmp2")
                nc.vector.tensor_scalar(tmp2[:], ps_f[:], fb2_[:], None,
                                        op0=OP.add)
                nc.vector.tensor_add(sl, sl, tmp2[:])

        if stage <= 5:
            dbg_out(q_T)
            return
        # ---------------- final layernorm + head ----------------
        hw_t = wp.tile([E, 37], F32, tag="hw")
        dma(out=hw_t[:], in_=ins["head_w"])
        gf = load_col("lnf_g", None, P, "gf")
        bf = load_col("lnf_b", None, P, "bf")
        hb = wp.tile([37, 1], F32, tag="hb")
        dma(out=hb[:], in_=ins["head_b"])
        hw_e = wp.tile([E, 37], F32, tag="hwe")
        nc.vector.tensor_scalar_mul(hw_e[:], hw_t[:], gf[:])
        ps4 = psum(p_q, [37, 1])
        nc.tensor.matmul(ps4[:], hw_t[:], bf[:])
        hbe = wp.tile([37, 1], F32, tag="hbe")
        nc.vector.tensor_add(hbe[:], ps4[:], hb[:])
        qxf = ln_cols(q_T[:], BC, "lf")
        ps_o = psum(p_sc, [37, BC])
        nc.tensor.matmul(ps_o[:], hw_e[:], qxf[:])
        out_sb = work.tile([37, BC], F32, tag="osb")
        nc.scalar.activation(out_sb[:], ps_o[:], AF.Identity, bias=hbe[:])
        dma(out=out_ap.rearrange("b o -> o b"), in_=out_sb[:])


def input_specs_for(BC):
    full = [
        ("enc_out", [BC, T, E]), ("x1", [BC, 3, 37, 1]), ("x2", [BC, 7, 4, 1]),
        ("x3", [BC, 4]),
        ("c11_w", [8, 3, 3, 3]), ("c11_b", [8]), ("bn11_g", [8]), ("bn11_b", [8]),
        ("c12_w", [8, 8, 3, 3]), ("c12_b", [8]), ("bn12_g", [8]), ("bn12_b", [8]),
        ("fc1_w", [296, 64]), ("fc1_b", [64]),
        ("c21_w", [8, 7, 3, 3]), ("c21_b", [8]), ("bn21_g", [8]), ("bn21_b", [8]),
        ("fc2_w", [32, 16]), ("fc2_b", [16]),
        ("fc_w", [84, 128]), ("fc_b", [128]),
        ("wk", [L, E, E]), ("wq", [L, E, E]), ("wv", [L, E, E]),
        ("proj_w", [L, E, E]), ("proj_b", [L, E]),
        ("ln1_g", [L, E]), ("ln1_b", [L, E]), ("ln2_g", [L, E]), ("ln2_b", [L, E]),
        ("ln3_g", [L, E]), ("ln3_b", [L, E]), ("ln4_g", [L, E]), ("ln4_b", [L, E]),
        ("ff_w1", [L, E, 4 * E]), ("ff_b1", [L, 4 * E]),
        ("ff_w2", [L, 4 * E, E]), ("ff_b2", [L, E]),
        ("lnf_g", [E]), ("lnf_b", [E]), ("head_w", [E, 37]), ("head_b", [37]),
    ]
    return [(n, s, F32) for n, s in full]


def build_program(BC=256, stage=99):
    """Build the Bass program for one core handling BC batch elements."""
    nc = bacc.Bacc("TRN2", target_bir_lowering=False, debug=False,
                   enable_asserts=True, num_devices=1)
    ins = {}
    for name, shape, dt_ in input_specs_for(BC):
        ins[name] = nc.dram_tensor(name, shape, dt_, kind="ExternalInput").ap()
    out_ap = nc.dram_tensor("out", [BC, 37], F32, kind="ExternalOutput").ap()
    with tile.TileContext(nc) as tc:
        with ExitStack() as ctx:
            decoder_body(ctx, tc, out_ap, ins, BC, stage=stage)
    nc.compile()
    return nc


_prog_cache = {}


def kernel(**inputs):
    BC = B_FULL // N_CORES
    if BC not in _prog_cache:
        _prog_cache[BC] = build_program(BC)
    nc = _prog_cache[BC]
    in_maps = []
    for c in range(N_CORES):
        m = {}
        for name, shape, _ in input_specs_for(BC):
            arr = np.ascontiguousarray(np.asarray(inputs[name], dtype=np.float32))
            if name in SHARDED:
                arr = arr[c * BC:(c + 1) * BC]
            m[name] = np.ascontiguousarray(arr)
        in_maps.append(m)
    res = run_bass_kernel_spmd(nc, in_maps, core_ids=list(range(N_CORES)))
    return np.concatenate([r["out"] for r in res.results], axis=0)

